# revision 3
# baseline (speedup 1.0000x reference)
"""LittleBitLinear Trainium2 kernel.

Computation (per pathway):  Y = (((x*g) @ sign(V)) * l) @ sign(U)^T * h
out = pathway_primary + pathway_residual + bias

Strategy:
  - Data-parallel over tokens: 8192 tokens -> 8 cores x 1024 tokens. No collectives.
  - All scale vectors fold into the sign matrices on host:
        W1 = g[:,None] * sign(V)            [Din, R]
        W2 = l[:,None] * sign(U).T * h      [R, Dout]
    so per core:  out_shard = x_shard @ W1_p @ W2_p + x_shard @ W1_r @ W2_r + bias
  - Work in transposed token space on device (out^T = W2^T @ (W1^T @ x^T)):
    both matmul stages then take naturally-laid-out stationary (lhsT) tiles and
    the phase-1 output feeds phase-2 as the moving operand with no transposes.
  - bf16 matmuls (sign matrices are exactly +-1 in bf16), fp32 PSUM accumulate.
"""

import sys

import numpy as np

for _p in ("/opt/trn_rl_repo",):
    if _p not in sys.path:
        sys.path.insert(0, _p)

import ml_dtypes

TOKENS, D_IN, D_OUT, RANK = 8192, 4096, 4096, 1024
N_CORES = 8
T_CORE = TOKENS // N_CORES            # 1024 tokens per core
P = 128                               # partitions
NT = 512                              # matmul free-dim chunk (one PSUM bank)
N_TCH = T_CORE // NT                  # 2 token chunks per core
N_DT = D_IN // P                      # 32 contraction tiles, phase 1
N_RT = RANK // P                      # 8 rank tiles
N_OT = D_OUT // P                     # 32 output tiles

BF16 = ml_dtypes.bfloat16

_CACHE = {}


def _build_program():
    import concourse.bass as bass
    import concourse.mybir as mybir
    import concourse.tile as tile
    from concourse import bacc

    dt = mybir.dt

    nc = bacc.Bacc(
        "TRN2",
        target_bir_lowering=False,
        debug=False,
        enable_asserts=False,
    )

    # Inputs.  Host layouts are pre-tiled so every DMA is contiguous,
    # partition-major.
    xT_d = nc.dram_tensor("xT", [P, N_DT, T_CORE], dt.bfloat16, kind="ExternalInput")
    w1_d = [
        nc.dram_tensor(f"w1_{p}", [N_RT, P, N_DT, P], dt.bfloat16, kind="ExternalInput")
        for p in range(2)
    ]
    w2_d = [
        nc.dram_tensor(f"w2_{p}", [N_OT, P, N_RT, P], dt.bfloat16, kind="ExternalInput")
        for p in range(2)
    ]
    bias_d = nc.dram_tensor("bias", [P, N_OT], dt.float32, kind="ExternalInput")
    out_d = nc.dram_tensor("outT", [D_OUT, T_CORE], dt.float32, kind="ExternalOutput")

    with tile.TileContext(nc) as tc:
        with (
            tc.tile_pool(name="xres", bufs=1) as xpool,
            tc.tile_pool(name="yres", bufs=1) as ypool,
            tc.tile_pool(name="w1s", bufs=3) as w1pool,
            tc.tile_pool(name="w2s", bufs=3) as w2pool,
            tc.tile_pool(name="ostage", bufs=4) as opool,
            tc.tile_pool(name="psum", bufs=4, space=bass.MemorySpace.PSUM) as pspool,
            tc.tile_pool(name="misc", bufs=1) as mpool,
        ):
            bias_sb = mpool.tile([P, N_OT], dt.float32, tag="bias")
            nc.sync.dma_start(bias_sb[:], bias_d[:])

            # Resident x^T, loaded per contraction tile so matmuls can start
            # as soon as the first slice lands.
            xT_sb = xpool.tile([P, N_DT, T_CORE], dt.bfloat16, tag="xT")
            for dti in range(N_DT):
                nc.sync.dma_start(xT_sb[:, dti, :], xT_d[:, dti, :])

            y_sb = [
                ypool.tile([P, N_RT, T_CORE], dt.bfloat16, tag=f"y{p}", name=f"y{p}")
                for p in range(2)
            ]

            # ---- Phase 1:  Y_p[r, t] = sum_d W1_p[d, r] * xT[d, t] ----
            for p in range(2):
                for rt in range(N_RT):
                    w1_sb = w1pool.tile([P, N_DT, P], dt.bfloat16, tag="w1")
                    nc.sync.dma_start(w1_sb[:], w1_d[p][rt])
                    for tch in range(N_TCH):
                        ps = pspool.tile([P, NT], dt.float32, tag="ps")
                        for dti in range(N_DT):
                            nc.tensor.matmul(
                                ps[:],
                                w1_sb[:, dti, :],
                                xT_sb[:, dti, tch * NT : (tch + 1) * NT],
                                start=(dti == 0),
                                stop=(dti == N_DT - 1),
                            )
                        nc.vector.tensor_copy(
                            y_sb[p][:, rt, tch * NT : (tch + 1) * NT], ps[:]
                        )

            # ---- Phase 2:  outT[o, t] = sum_p sum_r W2_p[r, o] * Y_p[r, t] + bias[o]
            for ot in range(N_OT):
                w2_sb = []
                for p in range(2):
                    w = w2pool.tile(
                        [P, N_RT, P], dt.bfloat16, tag=f"w2_{p}", name=f"w2sb_{p}"
                    )
                    nc.sync.dma_start(w[:], w2_d[p][ot])
                    w2_sb.append(w)
                for tch in range(N_TCH):
                    ps = pspool.tile([P, NT], dt.float32, tag="ps")
                    for p in range(2):
                        for rt in range(N_RT):
                            nc.tensor.matmul(
                                ps[:],
                                w2_sb[p][:, rt, :],
                                y_sb[p][:, rt, tch * NT : (tch + 1) * NT],
                                start=(p == 0 and rt == 0),
                                stop=(p == 1 and rt == N_RT - 1),
                            )
                    o_sb = opool.tile([P, NT], dt.float32, tag="ost")
                    nc.vector.tensor_scalar_add(o_sb[:], ps[:], bias_sb[:, ot : ot + 1])
                    nc.sync.dma_start(
                        out_d[ot * P : (ot + 1) * P, tch * NT : (tch + 1) * NT], o_sb[:]
                    )

    nc.compile()
    return nc


def _get_program():
    if "nc" not in _CACHE:
        _CACHE["nc"] = _build_program()
    return _CACHE["nc"]


def _prep_weights(U, V, h, l, g):
    """W1 = g[:,None]*sign(V)  [Din,R];  W2 = l[:,None]*sign(U).T*h  [R,Dout].
    Returned pre-tiled for contiguous partition-major DMA."""
    W1 = (g[:, None] * np.sign(V)).astype(BF16)
    W2 = (l[:, None] * np.sign(U).T * h[None, :]).astype(BF16)
    # W1[d, r] -> [rt, d_i, dt, r_i]
    w1t = np.ascontiguousarray(
        W1.reshape(N_DT, P, N_RT, P).transpose(2, 1, 0, 3)
    )
    # W2[r, o] -> [ot, r_i, rt, o_i]
    w2t = np.ascontiguousarray(
        W2.reshape(N_RT, P, N_OT, P).transpose(2, 1, 0, 3)
    )
    return w1t, w2t


def kernel(
    x,
    U_primary,
    V_primary,
    h_primary,
    l_primary,
    g_primary,
    U_residual,
    V_residual,
    h_residual,
    l_residual,
    g_residual,
    bias,
    _want_trace=False,
):
    from concourse.bass_utils import run_bass_kernel_spmd

    x = np.asarray(x, dtype=np.float32)
    w1p, w2p = _prep_weights(
        np.asarray(U_primary), np.asarray(V_primary),
        np.asarray(h_primary), np.asarray(l_primary), np.asarray(g_primary),
    )
    w1r, w2r = _prep_weights(
        np.asarray(U_residual), np.asarray(V_residual),
        np.asarray(h_residual), np.asarray(l_residual), np.asarray(g_residual),
    )
    bias_h = np.ascontiguousarray(
        np.asarray(bias, dtype=np.float32).reshape(N_OT, P).T
    )

    in_maps = []
    for c in range(N_CORES):
        xs = x[c * T_CORE : (c + 1) * T_CORE]          # [T_CORE, Din]
        # x^T tiled: [d_i, dt, t]
        xt = np.ascontiguousarray(
            xs.T.reshape(N_DT, P, T_CORE).transpose(1, 0, 2)
        ).astype(BF16)
        in_maps.append(
            {
                "xT": xt,
                "w1_0": w1p, "w1_1": w1r,
                "w2_0": w2p, "w2_1": w2r,
                "bias": bias_h,
            }
        )

    nc = _get_program()
    res = run_bass_kernel_spmd(
        nc, in_maps, core_ids=list(range(N_CORES)), trace=_want_trace
    )
    if _want_trace:
        _CACHE["last_result"] = res

    out = np.empty((TOKENS, D_OUT), dtype=np.float32)
    for c in range(N_CORES):
        out[c * T_CORE : (c + 1) * T_CORE] = res.results[c]["outT"].T
    return out
